# revision 14
# baseline (speedup 1.0000x reference)
"""Cross-attention head (B=4, T=S=4096, C=1024, HS=64) on 8 TRN2 NeuronCores.

Sharding: core i handles batch b = i//2, query-half th = i%2 (2048 query rows).
Each core gets a transposed slice xT [C, 2048] and its batch's encT [C, S]
(host-side layout prep, cast to bf16), plus packed weights Wqq=[Wq|Wq],
Wkv=[Wk|Wv], Wvk=[Wv|Wk] ([C,128] each, bf16).

Per-core pipeline (bf16 matmul operands, fp32 PSUM accumulation):
  qT2 [128, 2048] = (Wqq)^T @ xT           rows 0:64 = q^T, rows 64:128 = copy
  stream over s in 512-chunks, alternating Wkv / Wvk so that k^T lands on
  partitions 0:64 (even chunks) or 64:128 (odd chunks); v^T on the other half
  of a single kv tile (one PSUM->SBUF cast-copy per chunk).
  v^T chunks are transposed on the PE (identity matmul) into v_aug [128s, 65]
  tiles (col 64 = 1.0, giving the softmax denominator for free).
  scoresT [s,t] = kT^T_block @ qT2: two row-packed matmuls (PE rows 0:63 and
  64:127, concurrent quadrants); one ACT Exp (scale=1/8, bf16 out) evacuates
  both PSUM banks.
  PV (transposed): po[t-block 128, 65] += U_block^T @ v_aug, accumulated over
  all 32 s-blocks; lhsT = U t-block is a full 128-col bf16 weight (FWL), N=65.
  Tail (DVE-only): out[t, d] = po[t, tb, 0:64] * recip(po[t, tb, 64]).
"""

import numpy as np
import ml_dtypes

B, T, S, C, HS = 4, 4096, 4096, 1024, 64
NCORE = 8
TSH = T // 2            # 2048 query rows per core
KT = C // 128           # 8 contraction k-tiles
NTCH = TSH // 512       # 4 t-chunks
NCP = S // 1024         # 4 s-chunk pairs (each pair = 2x 512 keys)
SCALE = HS ** -0.5
BF16 = ml_dtypes.bfloat16

_CACHE = {}


def _build(reps=1):
    import concourse.bass as bass
    import concourse.mybir as mybir
    from concourse import bacc
    from concourse.tile import TileContext
    from concourse.masks import make_identity

    f32 = mybir.dt.float32
    bf16 = mybir.dt.bfloat16
    Exp = mybir.ActivationFunctionType.Exp

    nc = bacc.Bacc("TRN2", target_bir_lowering=False, debug=False,
                   num_devices=NCORE)
    # Host-packed layouts: every DMA sees per-partition contiguous runs.
    # xT packed as [tch, p, k, 512], encT as [sch, p, k, 512],
    # weights as [p, k, 128], out as [tch, p, j, 64].
    xT = nc.dram_tensor("xT", [NTCH, 128, KT, 512], bf16, kind="ExternalInput")
    encT = nc.dram_tensor("encT", [S // 512, 128, KT, 512], bf16,
                          kind="ExternalInput")
    wqq = nc.dram_tensor("Wqq", [128, KT, 128], bf16, kind="ExternalInput")
    wkv = nc.dram_tensor("Wkv", [128, KT, 128], bf16, kind="ExternalInput")
    wvk = nc.dram_tensor("Wvk", [128, KT, 128], bf16, kind="ExternalInput")
    out = nc.dram_tensor("out", [NTCH, 128, 4, HS], f32,
                         kind="ExternalOutput")

    xT_v = xT[:]       # [4, 128, 8, 512]
    encT_v = encT[:]   # [8, 128, 8, 512]
    out_v = out[:]     # [4, 128, 4, 64]

    with TileContext(nc) as tc:
        from contextlib import ExitStack
        with ExitStack() as ctx:
            ep = ctx.enter_context
            wpool = ep(tc.tile_pool(name="w", bufs=1))
            qpool = ep(tc.tile_pool(name="qt", bufs=2))
            xtp = ep(tc.tile_pool(name="xt", bufs=3))
            encp = ep(tc.tile_pool(name="enc", bufs=4))
            kvp = ep(tc.tile_pool(name="kv", bufs=4))
            vap = ep(tc.tile_pool(name="va", bufs=4))
            up = ep(tc.tile_pool(name="u", bufs=3))
            obp = ep(tc.tile_pool(name="ob", bufs=2))
            rp = ep(tc.tile_pool(name="r", bufs=2))
            # PSUM: po 4 banks + shared transient pool 2x[128,1024] = 4 -> 8
            pop = ep(tc.tile_pool(name="po", bufs=1, space="PSUM"))
            psp = ep(tc.tile_pool(name="ps", bufs=2, space="PSUM"))

            # static tiles
            ident = wpool.tile([128, 128], bf16, tag="ident")
            make_identity(nc, ident[:])
            w_sb = {}
            for name, dram in (("qq", wqq), ("kv", wkv), ("vk", wvk)):
                wt = wpool.tile([128, KT * 128], bf16, tag=f"w{name}")
                nc.sync.dma_start(
                    out=wt[:].rearrange("p (k m) -> p k m", k=KT),
                    in_=dram[:])
                w_sb[name] = wt[:].rearrange("p (k m) -> p k m", k=KT)

            for _rep in range(reps):
                qt2 = qpool.tile([128, TSH], bf16, tag="qt2")
                # po[tch]: [t=128, 4 t-blocks x (64 dims + Z)] accumulators
                po = [pop.tile([128, 4 * 65], f32, tag=f"po{t}", name=f"po{t}")
                      for t in range(NTCH)]

                def make_kv_thunks(cp):
                    """Emit-later closures for loading/projecting s-chunk pair
                    cp.  Returns (thunks, kv_tiles, va_views)."""
                    kvs, vas = [None, None], [None, None]
                    thunks = []

                    def load(par):
                        def f():
                            sch = 2 * cp + par
                            enc = encp.tile([128, KT * 512], bf16, tag="enc")
                            enc3 = enc[:].rearrange("p (k n) -> p k n", k=KT)
                            nc.sync.dma_start(out=enc3, in_=encT_v[sch])
                            pkv = psp.tile([128, 512], f32, tag="ps",
                                           name="pkv")
                            wname = "kv" if par == 0 else "vk"
                            for k in range(KT):
                                nc.tensor.matmul(pkv[:], w_sb[wname][:, k, :],
                                                 enc3[:, k, :],
                                                 start=(k == 0),
                                                 stop=(k == KT - 1))
                            # par 0: kT rows 0:64, vT rows 64:128
                            # par 1: vT rows 0:64, kT rows 64:128
                            kv = kvp.tile([128, 512], bf16, tag="kv")
                            nc.vector.tensor_copy(kv[:], pkv[:])
                            va = vap.tile([128, 4 * 65], bf16, tag="va")
                            va3 = va[:].rearrange("p (j m) -> p j m", j=4)
                            nc.gpsimd.memset(va3[:, :, 64:65], 1.0)
                            kvs[par] = kv
                            vas[par] = va3
                        return f

                    def vtrans(par, j):
                        def f():
                            va3 = vas[par]
                            kv = kvs[par]
                            rows = slice(64, 128) if par == 0 else slice(0, 64)
                            pvt = psp.tile([128, 65], f32, tag="ps",
                                           name="pvt")
                            nc.tensor.matmul(pvt[:, 0:64],
                                             kv[rows, j * 128:(j + 1) * 128],
                                             ident[rows, rows],
                                             start=True, stop=True)
                            nc.vector.tensor_copy(va3[:, j, 0:64],
                                                  pvt[:, 0:64])
                        return f

                    for par in range(2):
                        thunks.append(load(par))
                        for j in range(4):
                            thunks.append(vtrans(par, j))
                    return thunks, kvs, vas

                def emit_tail(tch):
                    """Normalize + store po[tch] (emitted right after its
                    last PV so the tail overlaps the remaining s-stream).
                    po is t-major, so the tail is DVE-only."""
                    po3 = po[tch][:].rearrange("p (j m) -> p j m", j=4)
                    ob = obp.tile([128, 4 * 64], f32, tag="ob")
                    ob3 = ob[:].rearrange("p (j d) -> p j d", j=4)
                    for j in range(4):
                        r = rp.tile([128, 1], f32, tag="r")
                        nc.vector.reciprocal(r[:], po3[:, j, 64:65])
                        nc.vector.tensor_scalar_mul(ob3[:, j, :],
                                                    po3[:, j, 0:64], r[:])
                    nc.sync.dma_start(out=out_v[tch], in_=ob3)

                def emit_pv(prev):
                    """PV matmuls for a previously-exp'd pair (one-pair SW
                    pipeline keeps the PE from stalling on the current exp).
                    Transposed form: out[t-block, 65] = U_block^T @ v_aug;
                    lhsT is a full 128-col bf16 weight load (FWL-eligible),
                    N=65 stream, plain serial PSUM accumulation."""
                    u, pvas, pcp, ptch, ppb = prev
                    first = (pcp == 0 and ppb == 0)
                    last = (pcp == NCP - 1 and ppb == 3)
                    po3 = po[ptch][:].rearrange("p (j m) -> p j m", j=4)
                    # start=True zeroes the whole PSUM zero-region (bank),
                    # so it must be emitted exactly once per po bank: on the
                    # first matmul only.  Later tb-regions rely on the
                    # pending-zero left by that single start.
                    for h in range(2):
                        va3 = pvas[h]
                        for tb in range(4):
                            nc.tensor.matmul(
                                po3[:, tb, :],
                                u[:, 512 * h + 128 * tb:
                                   512 * h + 128 * (tb + 1)],
                                va3[:, ppb, :],
                                start=(first and h == 0 and tb == 0),
                                stop=(last and h == 1 and tb == 3),
                                skip_group_check=True)
                    if last:
                        emit_tail(ptch)

                # ---- Phase Q interleaved with s-chunk pair 0 load
                cur = make_kv_thunks(0)
                kv0_sched = {0: [0], 1: [5], 2: [1, 2, 3, 4],
                             3: [6, 7, 8, 9]}
                for tch in range(NTCH):
                    xt = xtp.tile([128, KT * 512], bf16, tag="xt")
                    xt3 = xt[:].rearrange("p (k n) -> p k n", k=KT)
                    nc.sync.dma_start(out=xt3, in_=xT_v[tch])
                    pq = psp.tile([128, 512], f32, tag="ps", name="pq")
                    for k in range(KT):
                        nc.tensor.matmul(pq[:], w_sb["qq"][:, k, :],
                                         xt3[:, k, :],
                                         start=(k == 0), stop=(k == KT - 1))
                    nc.vector.tensor_copy(
                        qt2[:, tch * 512:(tch + 1) * 512], pq[:])
                    for ti0 in kv0_sched[tch]:
                        cur[0][ti0]()
                prev = None
                for cp in range(NCP):
                    _, kvs, vas = cur
                    nxt = make_kv_thunks(cp + 1) if cp + 1 < NCP else ([], None, None)
                    n_thunks = len(nxt[0])
                    ti = 0
                    pair_idx = 0
                    for tch in range(NTCH):
                        for pb in range(4):
                            ps = psp.tile([128, 1024], f32, tag="ps")
                            nc.tensor.matmul(
                                ps[:, 0:512],
                                kvs[0][0:64, pb * 128:(pb + 1) * 128],
                                qt2[0:64, tch * 512:(tch + 1) * 512],
                                start=True, stop=True)
                            nc.tensor.matmul(
                                ps[:, 512:1024],
                                kvs[1][64:128, pb * 128:(pb + 1) * 128],
                                qt2[64:128, tch * 512:(tch + 1) * 512],
                                start=True, stop=True)
                            u = up.tile([128, 1024], bf16, tag="u")
                            nc.scalar.activation(u[:], ps[:], Exp, scale=SCALE)
                            if prev is not None:
                                emit_pv(prev)
                            prev = (u, (vas[0], vas[1]), cp, tch, pb)
                            pair_idx += 1
                            # interleave next chunk-pair's kv work
                            target = (n_thunks * pair_idx) // 16
                            while ti < target:
                                nxt[0][ti]()
                                ti += 1
                    while ti < n_thunks:
                        nxt[0][ti]()
                        ti += 1
                    cur = nxt
                emit_pv(prev)

    nc.compile()
    return nc


def _get_nc(reps=1):
    if reps not in _CACHE:
        _CACHE[reps] = _build(reps)
    return _CACHE[reps]


def _pack_act(a, nch):
    """[L, C] row-major -> [L/512, 128, KT, 512] (chunk, partition, k, col)."""
    return np.ascontiguousarray(
        a.astype(BF16).reshape(nch, 512, KT, 128).transpose(0, 3, 2, 1))


def _pack_w(w2):
    """[C, 128] -> [128, KT, 128]."""
    return np.ascontiguousarray(
        w2.astype(BF16).reshape(KT, 128, 128).transpose(1, 0, 2))


def _prep_inputs(x, encode_out, Wq, Wk, Wv):
    x = np.asarray(x, dtype=np.float32)
    encode_out = np.asarray(encode_out, dtype=np.float32)
    Wq = np.asarray(Wq, dtype=np.float32)
    Wk = np.asarray(Wk, dtype=np.float32)
    Wv = np.asarray(Wv, dtype=np.float32)
    wqq = _pack_w(np.concatenate([Wq, Wq], axis=1))
    wkv = _pack_w(np.concatenate([Wk, Wv], axis=1))
    wvk = _pack_w(np.concatenate([Wv, Wk], axis=1))
    encTs = [_pack_act(encode_out[b], S // 512) for b in range(B)]
    in_maps = []
    for core in range(NCORE):
        b, th = divmod(core, 2)
        xTi = _pack_act(x[b, th * TSH:(th + 1) * TSH, :], NTCH)
        in_maps.append({"xT": xTi, "encT": encTs[b],
                        "Wqq": wqq, "Wkv": wkv, "Wvk": wvk})
    return in_maps


def kernel(x, encode_out, Wq, Wk, Wv):
    from concourse.bass_utils import run_bass_kernel_spmd
    nc = _get_nc(1)
    in_maps = _prep_inputs(x, encode_out, Wq, Wk, Wv)
    res = run_bass_kernel_spmd(nc, in_maps, list(range(NCORE)))
    out = np.empty((B, T, HS), dtype=np.float32)
    for core in range(NCORE):
        b, th = divmod(core, 2)
        o = res.results[core]["out"]            # [4, 128, 4, 64]
        out[b, th * TSH:(th + 1) * TSH] = (
            o.transpose(0, 2, 1, 3).reshape(TSH, HS))
    return out


# revision 17
# speedup vs baseline: 1.2575x; 1.2575x over previous
"""Cross-attention head (B=4, T=S=4096, C=1024, HS=64) on 8 TRN2 NeuronCores.

Sharding: core i handles batch b = i//2, query-half th = i%2 (2048 query rows).
Each core gets a transposed slice xT [C, 2048] and its batch's encT [C, S]
(host-side layout prep, cast to bf16), plus packed weights Wqq=[Wq|Wq],
Wkv=[Wk|Wv], Wvk=[Wv|Wk] ([C,128] each, bf16).

Per-core pipeline (bf16 matmul operands, fp32 PSUM accumulation):
  qT2 [128, 2048] = (Wqq)^T @ xT           rows 0:64 = q^T, rows 64:128 = copy
  stream over s in 512-chunks, alternating Wkv / Wvk so that k^T lands on
  partitions 0:64 (even chunks) or 64:128 (odd chunks); v^T on the other half
  of a single kv tile (one PSUM->SBUF cast-copy per chunk).
  v^T chunks are transposed on the PE (identity matmul) into v_aug [128s, 65]
  tiles (col 64 = 1.0, giving the softmax denominator for free).
  scoresT [s,t] = kT^T_block @ qT2: two row-packed matmuls (PE rows 0:63 and
  64:127, concurrent quadrants); one ACT Exp (scale=1/8, bf16 out) evacuates
  both PSUM banks.
  PV (transposed): po[t-block 128, 65] += U_block^T @ v_aug, accumulated over
  all 32 s-blocks; lhsT = U t-block is a full 128-col bf16 weight (FWL), N=65.
  Tail (DVE-only): out[t, d] = po[t, tb, 0:64] * recip(po[t, tb, 64]).
"""

import numpy as np
import ml_dtypes

B, T, S, C, HS = 4, 4096, 4096, 1024, 64
NCORE = 8
TSH = T // 2            # 2048 query rows per core
KT = C // 128           # 8 contraction k-tiles
NTCH = TSH // 512       # 4 t-chunks
NCP = S // 1024         # 4 s-chunk pairs (each pair = 2x 512 keys)
SCALE = HS ** -0.5
BF16 = ml_dtypes.bfloat16

_CACHE = {}


def _build(reps=1):
    import concourse.bass as bass
    import concourse.mybir as mybir
    from concourse import bacc
    from concourse.tile import TileContext
    from concourse.masks import make_identity

    f32 = mybir.dt.float32
    bf16 = mybir.dt.bfloat16
    Exp = mybir.ActivationFunctionType.Exp

    nc = bacc.Bacc("TRN2", target_bir_lowering=False, debug=False,
                   num_devices=NCORE)
    # Host-packed layouts: every DMA sees per-partition contiguous runs.
    # xT packed as [tch, p, k, 512], encT as [sch, p, k, 512],
    # weights as [p, k, 128], out as [tch, p, j, 64].
    xT = nc.dram_tensor("xT", [NTCH, 128, KT, 512], bf16, kind="ExternalInput")
    encT = nc.dram_tensor("encT", [S // 512, 128, KT, 512], bf16,
                          kind="ExternalInput")
    wqq = nc.dram_tensor("Wqq", [128, KT, 128], bf16, kind="ExternalInput")
    wkv = nc.dram_tensor("Wkv", [128, KT, 128], bf16, kind="ExternalInput")
    wvk = nc.dram_tensor("Wvk", [128, KT, 128], bf16, kind="ExternalInput")
    out = nc.dram_tensor("out", [NTCH, 128, 4, HS], f32,
                         kind="ExternalOutput")

    xT_v = xT[:]       # [4, 128, 8, 512]
    encT_v = encT[:]   # [8, 128, 8, 512]
    out_v = out[:]     # [4, 128, 4, 64]

    with TileContext(nc) as tc:
        from contextlib import ExitStack
        with ExitStack() as ctx:
            ep = ctx.enter_context
            wpool = ep(tc.tile_pool(name="w", bufs=1))
            qpool = ep(tc.tile_pool(name="qt", bufs=2))
            xtp = ep(tc.tile_pool(name="xt", bufs=4))
            encp = ep(tc.tile_pool(name="enc", bufs=6))
            kvp = ep(tc.tile_pool(name="kv", bufs=6))
            vap = ep(tc.tile_pool(name="va", bufs=6))
            up = ep(tc.tile_pool(name="u", bufs=4))
            obp = ep(tc.tile_pool(name="ob", bufs=2))
            rp = ep(tc.tile_pool(name="r", bufs=2))
            # PSUM: po 4 banks + shared transient pool 2x[128,1024] = 4 -> 8
            pop = ep(tc.tile_pool(name="po", bufs=1, space="PSUM"))
            psp = ep(tc.tile_pool(name="ps", bufs=2, space="PSUM"))

            # static tiles
            ident = wpool.tile([128, 128], bf16, tag="ident")
            make_identity(nc, ident[:])
            w_sb = {}
            for name, dram in (("qq", wqq), ("kv", wkv), ("vk", wvk)):
                wt = wpool.tile([128, KT * 128], bf16, tag=f"w{name}")
                nc.sync.dma_start(
                    out=wt[:].rearrange("p (k m) -> p k m", k=KT),
                    in_=dram[:])
                w_sb[name] = wt[:].rearrange("p (k m) -> p k m", k=KT)

            for _rep in range(reps):
                qt2 = qpool.tile([128, TSH], bf16, tag="qt2")
                # po[tch]: [t=128, 4 t-blocks x (64 dims + Z)] accumulators
                po = [pop.tile([128, 4 * 65], f32, tag=f"po{t}", name=f"po{t}")
                      for t in range(NTCH)]

                def make_kv_thunks(cp):
                    """Emit-later closures for loading/projecting s-chunk pair
                    cp.  Returns (thunks, kv_tiles, va_views)."""
                    kvs, vas = [None, None], [None, None]
                    thunks = []

                    def load(par):
                        def f():
                            sch = 2 * cp + par
                            enc = encp.tile([128, KT * 512], bf16, tag="enc")
                            enc3 = enc[:].rearrange("p (k n) -> p k n", k=KT)
                            nc.sync.dma_start(out=enc3, in_=encT_v[sch])
                            pkv = psp.tile([128, 512], f32, tag="ps",
                                           name="pkv")
                            wname = "kv" if par == 0 else "vk"
                            for k in range(KT):
                                nc.tensor.matmul(pkv[:], w_sb[wname][:, k, :],
                                                 enc3[:, k, :],
                                                 start=(k == 0),
                                                 stop=(k == KT - 1))
                            # par 0: kT rows 0:64, vT rows 64:128
                            # par 1: vT rows 0:64, kT rows 64:128
                            kv = kvp.tile([128, 512], bf16, tag="kv")
                            nc.vector.tensor_copy(kv[:], pkv[:])
                            va = vap.tile([128, 4 * 65], bf16, tag="va")
                            va3 = va[:].rearrange("p (j m) -> p j m", j=4)
                            nc.gpsimd.memset(va3[:, :, 64:65], 1.0)
                            kvs[par] = kv
                            vas[par] = va3
                        return f

                    def vtrans(par, j):
                        def f():
                            va3 = vas[par]
                            kv = kvs[par]
                            rows = slice(64, 128) if par == 0 else slice(0, 64)
                            pvt = psp.tile([128, 65], f32, tag="ps",
                                           name="pvt")
                            nc.tensor.matmul(pvt[:, 0:64],
                                             kv[rows, j * 128:(j + 1) * 128],
                                             ident[rows, rows],
                                             start=True, stop=True)
                            nc.vector.tensor_copy(va3[:, j, 0:64],
                                                  pvt[:, 0:64])
                        return f

                    # Both enc DMAs lead the interleave schedule so the DMA
                    # ring stays fed; the PE transposes trail them.
                    thunks.append(load(0))
                    thunks.append(load(1))
                    for par in range(2):
                        for j in range(4):
                            thunks.append(vtrans(par, j))
                    return thunks, kvs, vas

                def emit_tail(tch):
                    """Normalize + store po[tch] (emitted right after its
                    last PV so the tail overlaps the remaining s-stream).
                    po is t-major, so the tail is DVE-only."""
                    po3 = po[tch][:].rearrange("p (j m) -> p j m", j=4)
                    ob = obp.tile([128, 4 * 64], f32, tag="ob")
                    ob3 = ob[:].rearrange("p (j d) -> p j d", j=4)
                    for j in range(4):
                        r = rp.tile([128, 1], f32, tag="r")
                        nc.vector.reciprocal(r[:], po3[:, j, 64:65])
                        nc.vector.tensor_scalar_mul(ob3[:, j, :],
                                                    po3[:, j, 0:64], r[:])
                    nc.sync.dma_start(out=out_v[tch], in_=ob3)

                def emit_pv(prev):
                    """PV matmuls for a previously-exp'd pair (one-pair SW
                    pipeline keeps the PE from stalling on the current exp).
                    Transposed form: out[t-block, 65] = U_block^T @ v_aug;
                    lhsT is a full 128-col bf16 weight load (FWL-eligible),
                    N=65 stream, plain serial PSUM accumulation."""
                    u, pvas, pcp, ptch, ppb = prev
                    first = (pcp == 0 and ppb == 0)
                    last = (pcp == NCP - 1 and ppb == 3)
                    po3 = po[ptch][:].rearrange("p (j m) -> p j m", j=4)
                    # start=True zeroes the whole PSUM zero-region (bank),
                    # so it must be emitted exactly once per po bank: on the
                    # first matmul only.  Later tb-regions rely on the
                    # pending-zero left by that single start.
                    for h in range(2):
                        va3 = pvas[h]
                        for tb in range(4):
                            nc.tensor.matmul(
                                po3[:, tb, :],
                                u[:, 512 * h + 128 * tb:
                                   512 * h + 128 * (tb + 1)],
                                va3[:, ppb, :],
                                start=(first and h == 0 and tb == 0),
                                stop=(last and h == 1 and tb == 3),
                                skip_group_check=True)
                    if last:
                        emit_tail(ptch)

                # ---- Phase Q interleaved with s-chunk pair 0 load
                cur = make_kv_thunks(0)
                kv0_sched = {0: [0], 1: [1], 2: [2, 3, 4, 5],
                             3: [6, 7, 8, 9]}
                for tch in range(NTCH):
                    xt = xtp.tile([128, KT * 512], bf16, tag="xt")
                    xt3 = xt[:].rearrange("p (k n) -> p k n", k=KT)
                    nc.sync.dma_start(out=xt3, in_=xT_v[tch])
                    pq = psp.tile([128, 512], f32, tag="ps", name="pq")
                    for k in range(KT):
                        nc.tensor.matmul(pq[:], w_sb["qq"][:, k, :],
                                         xt3[:, k, :],
                                         start=(k == 0), stop=(k == KT - 1))
                    nc.vector.tensor_copy(
                        qt2[:, tch * 512:(tch + 1) * 512], pq[:])
                    for ti0 in kv0_sched[tch]:
                        cur[0][ti0]()
                prev = None
                for cp in range(NCP):
                    _, kvs, vas = cur
                    nxt = make_kv_thunks(cp + 1) if cp + 1 < NCP else ([], None, None)
                    n_thunks = len(nxt[0])
                    ti = 0
                    pair_idx = 0
                    for tch in range(NTCH):
                        for pb in range(4):
                            ps = psp.tile([128, 1024], f32, tag="ps")
                            nc.tensor.matmul(
                                ps[:, 0:512],
                                kvs[0][0:64, pb * 128:(pb + 1) * 128],
                                qt2[0:64, tch * 512:(tch + 1) * 512],
                                start=True, stop=True)
                            nc.tensor.matmul(
                                ps[:, 512:1024],
                                kvs[1][64:128, pb * 128:(pb + 1) * 128],
                                qt2[64:128, tch * 512:(tch + 1) * 512],
                                start=True, stop=True)
                            u = up.tile([128, 1024], bf16, tag="u")
                            nc.scalar.activation(u[:], ps[:], Exp, scale=SCALE)
                            if prev is not None:
                                emit_pv(prev)
                            prev = (u, (vas[0], vas[1]), cp, tch, pb)
                            pair_idx += 1
                            # interleave next chunk-pair's kv work
                            target = (n_thunks * pair_idx) // 16
                            while ti < target:
                                nxt[0][ti]()
                                ti += 1
                    while ti < n_thunks:
                        nxt[0][ti]()
                        ti += 1
                    cur = nxt
                emit_pv(prev)

    nc.compile()
    return nc


def _get_nc(reps=1):
    if reps not in _CACHE:
        _CACHE[reps] = _build(reps)
    return _CACHE[reps]


def _pack_act(a, nch):
    """[L, C] row-major -> [L/512, 128, KT, 512] (chunk, partition, k, col)."""
    return np.ascontiguousarray(
        a.astype(BF16).reshape(nch, 512, KT, 128).transpose(0, 3, 2, 1))


def _pack_w(w2):
    """[C, 128] -> [128, KT, 128]."""
    return np.ascontiguousarray(
        w2.astype(BF16).reshape(KT, 128, 128).transpose(1, 0, 2))


def _prep_inputs(x, encode_out, Wq, Wk, Wv):
    x = np.asarray(x, dtype=np.float32)
    encode_out = np.asarray(encode_out, dtype=np.float32)
    Wq = np.asarray(Wq, dtype=np.float32)
    Wk = np.asarray(Wk, dtype=np.float32)
    Wv = np.asarray(Wv, dtype=np.float32)
    wqq = _pack_w(np.concatenate([Wq, Wq], axis=1))
    wkv = _pack_w(np.concatenate([Wk, Wv], axis=1))
    wvk = _pack_w(np.concatenate([Wv, Wk], axis=1))
    encTs = [_pack_act(encode_out[b], S // 512) for b in range(B)]
    in_maps = []
    for core in range(NCORE):
        b, th = divmod(core, 2)
        xTi = _pack_act(x[b, th * TSH:(th + 1) * TSH, :], NTCH)
        in_maps.append({"xT": xTi, "encT": encTs[b],
                        "Wqq": wqq, "Wkv": wkv, "Wvk": wvk})
    return in_maps


def kernel(x, encode_out, Wq, Wk, Wv):
    from concourse.bass_utils import run_bass_kernel_spmd
    nc = _get_nc(1)
    in_maps = _prep_inputs(x, encode_out, Wq, Wk, Wv)
    res = run_bass_kernel_spmd(nc, in_maps, list(range(NCORE)))
    out = np.empty((B, T, HS), dtype=np.float32)
    for core in range(NCORE):
        b, th = divmod(core, 2)
        o = res.results[core]["out"]            # [4, 128, 4, 64]
        out[b, th * TSH:(th + 1) * TSH] = (
            o.transpose(0, 2, 1, 3).reshape(TSH, HS))
    return out


# revision 20
# speedup vs baseline: 1.4131x; 1.1237x over previous
"""Cross-attention head (B=4, T=S=4096, C=1024, HS=64) on 8 TRN2 NeuronCores.

Sharding: core i handles batch b = i//2, query-half th = i%2 (2048 query rows).
Each core gets a transposed slice xT [C, 2048] and its batch's encT [C, S]
(host-side layout prep, cast to bf16), plus packed weights Wqq=[Wq|Wq],
Wkv=[Wk|Wv], Wvk=[Wv|Wk] ([C,128] each, bf16).

Per-core pipeline (bf16 matmul operands, fp32 PSUM accumulation):
  qT2 [128, 2048] = (Wqq)^T @ xT           rows 0:64 = q^T, rows 64:128 = copy
  stream over s in 512-chunks, alternating Wkv / Wvk so that k^T lands on
  partitions 0:64 (even chunks) or 64:128 (odd chunks); v^T on the other half
  of a single kv tile (one PSUM->SBUF cast-copy per chunk).
  v^T chunks are transposed on the PE (identity matmul) into v_aug [128s, 65]
  tiles (col 64 = 1.0, giving the softmax denominator for free).
  scoresT [s,t] = kT^T_block @ qT2: two row-packed matmuls (PE rows 0:63 and
  64:127, concurrent quadrants); one ACT Exp (scale=1/8, bf16 out) evacuates
  both PSUM banks.
  PV (transposed): po[t-block 128, 65] += U_block^T @ v_aug, accumulated over
  all 32 s-blocks; lhsT = U t-block is a full 128-col bf16 weight (FWL), N=65.
  Tail (DVE-only): out[t, d] = po[t, tb, 0:64] * recip(po[t, tb, 64]).
"""

import numpy as np
import ml_dtypes

B, T, S, C, HS = 4, 4096, 4096, 1024, 64
NCORE = 8
TSH = T // 2            # 2048 query rows per core
KT = C // 128           # 8 contraction k-tiles
NTCH = TSH // 512       # 4 t-chunks
NCP = S // 1024         # 4 s-chunk pairs (each pair = 2x 512 keys)
SCALE = HS ** -0.5
BF16 = ml_dtypes.bfloat16

_CACHE = {}


def _build(reps=1):
    import concourse.bass as bass
    import concourse.mybir as mybir
    from concourse import bacc
    from concourse.tile import TileContext
    from concourse.masks import make_identity

    f32 = mybir.dt.float32
    bf16 = mybir.dt.bfloat16
    Exp = mybir.ActivationFunctionType.Exp

    nc = bacc.Bacc("TRN2", target_bir_lowering=False, debug=False,
                   num_devices=NCORE)
    # Host-packed layouts: every DMA sees per-partition contiguous runs.
    # xT packed as [tch, p, k, 512], encT as [sch, p, k, 512],
    # weights as [p, k, 128], out as [tch, p, j, 64].
    xT = nc.dram_tensor("xT", [NTCH, 128, KT, 512], bf16, kind="ExternalInput")
    encT = nc.dram_tensor("encT", [S // 512, 128, KT, 512], bf16,
                          kind="ExternalInput")
    wqq = nc.dram_tensor("Wqq", [128, KT, 128], bf16, kind="ExternalInput")
    wkv = nc.dram_tensor("Wkv", [128, KT, 128], bf16, kind="ExternalInput")
    wvk = nc.dram_tensor("Wvk", [128, KT, 128], bf16, kind="ExternalInput")
    out = nc.dram_tensor("out", [NTCH, 128, 4, HS], bf16,
                         kind="ExternalOutput")

    xT_v = xT[:]       # [4, 128, 8, 512]
    encT_v = encT[:]   # [8, 128, 8, 512]
    out_v = out[:]     # [4, 128, 4, 64]

    with TileContext(nc) as tc:
        from contextlib import ExitStack
        with ExitStack() as ctx:
            ep = ctx.enter_context
            wpool = ep(tc.tile_pool(name="w", bufs=1))
            qpool = ep(tc.tile_pool(name="qt", bufs=2))
            xtp = ep(tc.tile_pool(name="xt", bufs=4))
            encp = ep(tc.tile_pool(name="enc", bufs=6))
            kvp = ep(tc.tile_pool(name="kv", bufs=6))
            vap = ep(tc.tile_pool(name="va", bufs=6))
            up = ep(tc.tile_pool(name="u", bufs=4))
            obp = ep(tc.tile_pool(name="ob", bufs=2))
            rp = ep(tc.tile_pool(name="r", bufs=2))
            # PSUM: po 4 banks + shared transient pool 2x[128,1024] = 4 -> 8
            pop = ep(tc.tile_pool(name="po", bufs=1, space="PSUM"))
            psp = ep(tc.tile_pool(name="ps", bufs=2, space="PSUM"))

            # static tiles
            ident = wpool.tile([128, 128], bf16, tag="ident")
            make_identity(nc, ident[:])
            w_sb = {}
            for name, dram in (("qq", wqq), ("kv", wkv), ("vk", wvk)):
                wt = wpool.tile([128, KT * 128], bf16, tag=f"w{name}")
                nc.sync.dma_start(
                    out=wt[:].rearrange("p (k m) -> p k m", k=KT),
                    in_=dram[:])
                w_sb[name] = wt[:].rearrange("p (k m) -> p k m", k=KT)

            for _rep in range(reps):
                qt2 = qpool.tile([128, TSH], bf16, tag="qt2")
                # po[tch]: [t=128, 4 t-blocks x (64 dims + Z)] accumulators
                po = [pop.tile([128, 4 * 65], f32, tag=f"po{t}", name=f"po{t}")
                      for t in range(NTCH)]

                def make_kv_thunks(cp):
                    """Emit-later closures for loading/projecting s-chunk pair
                    cp.  Returns (thunks, kv_tiles, va_views)."""
                    kvs, vas = [None, None], [None, None]
                    thunks = []

                    def load(par):
                        def f():
                            sch = 2 * cp + par
                            enc = encp.tile([128, KT * 512], bf16, tag="enc")
                            enc3 = enc[:].rearrange("p (k n) -> p k n", k=KT)
                            nc.sync.dma_start(out=enc3, in_=encT_v[sch])
                            pkv = psp.tile([128, 512], f32, tag="ps",
                                           name="pkv")
                            wname = "kv" if par == 0 else "vk"
                            for k in range(KT):
                                nc.tensor.matmul(pkv[:], w_sb[wname][:, k, :],
                                                 enc3[:, k, :],
                                                 start=(k == 0),
                                                 stop=(k == KT - 1))
                            # par 0: kT rows 0:64, vT rows 64:128
                            # par 1: vT rows 0:64, kT rows 64:128
                            kv = kvp.tile([128, 512], bf16, tag="kv")
                            nc.vector.tensor_copy(kv[:], pkv[:])
                            va = vap.tile([128, 4 * 65], bf16, tag="va")
                            va3 = va[:].rearrange("p (j m) -> p j m", j=4)
                            nc.gpsimd.memset(va3[:, :, 64:65], 1.0)
                            kvs[par] = kv
                            vas[par] = va3
                        return f

                    def vtrans(par, j):
                        def f():
                            va3 = vas[par]
                            kv = kvs[par]
                            rows = slice(64, 128) if par == 0 else slice(0, 64)
                            pvt = psp.tile([128, 65], f32, tag="ps",
                                           name="pvt")
                            nc.tensor.matmul(pvt[:, 0:64],
                                             kv[rows, j * 128:(j + 1) * 128],
                                             ident[rows, rows],
                                             start=True, stop=True)
                            nc.vector.tensor_copy(va3[:, j, 0:64],
                                                  pvt[:, 0:64])
                        return f

                    # Both enc DMAs lead the interleave schedule so the DMA
                    # ring stays fed; the PE transposes trail them.
                    thunks.append(load(0))
                    thunks.append(load(1))
                    for par in range(2):
                        for j in range(4):
                            thunks.append(vtrans(par, j))
                    return thunks, kvs, vas

                def emit_tail(tch):
                    """Normalize + store po[tch] (emitted right after its
                    last PV so the tail overlaps the remaining s-stream).
                    po is t-major, so the tail is DVE-only."""
                    po3 = po[tch][:].rearrange("p (j m) -> p j m", j=4)
                    ob = obp.tile([128, 4 * 64], bf16, tag="ob")
                    ob3 = ob[:].rearrange("p (j d) -> p j d", j=4)
                    for j in range(4):
                        r = rp.tile([128, 1], f32, tag="r")
                        nc.vector.reciprocal(r[:], po3[:, j, 64:65])
                        nc.vector.tensor_scalar_mul(ob3[:, j, :],
                                                    po3[:, j, 0:64], r[:])
                    nc.sync.dma_start(out=out_v[tch], in_=ob3)

                def emit_pv(prev):
                    """PV matmuls for a previously-exp'd pair (one-pair SW
                    pipeline keeps the PE from stalling on the current exp).
                    Transposed form: out[t-block, 65] = U_block^T @ v_aug;
                    lhsT is a full 128-col bf16 weight load (FWL-eligible),
                    N=65 stream, plain serial PSUM accumulation."""
                    u, pvas, pcp, ptch, ppb = prev
                    first = (pcp == 0 and ppb == 0)
                    last = (pcp == NCP - 1 and ppb == 3)
                    po3 = po[ptch][:].rearrange("p (j m) -> p j m", j=4)
                    # start=True zeroes the whole PSUM zero-region (bank),
                    # so it must be emitted exactly once per po bank: on the
                    # first matmul only.  Later tb-regions rely on the
                    # pending-zero left by that single start.
                    for h in range(2):
                        va3 = pvas[h]
                        for tb in range(4):
                            nc.tensor.matmul(
                                po3[:, tb, :],
                                u[:, 512 * h + 128 * tb:
                                   512 * h + 128 * (tb + 1)],
                                va3[:, ppb, :],
                                start=(first and h == 0 and tb == 0),
                                stop=(last and h == 1 and tb == 3),
                                skip_group_check=True)
                    if last:
                        emit_tail(ptch)

                # ---- Phase Q interleaved with s-chunk pair 0 load
                cur = make_kv_thunks(0)
                kv0_sched = {0: [0], 1: [1], 2: [2, 3, 4, 5],
                             3: [6, 7, 8, 9]}
                for tch in range(NTCH):
                    xt = xtp.tile([128, KT * 512], bf16, tag="xt")
                    xt3 = xt[:].rearrange("p (k n) -> p k n", k=KT)
                    nc.sync.dma_start(out=xt3, in_=xT_v[tch])
                    pq = psp.tile([128, 512], f32, tag="ps", name="pq")
                    for k in range(KT):
                        nc.tensor.matmul(pq[:], w_sb["qq"][:, k, :],
                                         xt3[:, k, :],
                                         start=(k == 0), stop=(k == KT - 1))
                    nc.vector.tensor_copy(
                        qt2[:, tch * 512:(tch + 1) * 512], pq[:])
                    for ti0 in kv0_sched[tch]:
                        cur[0][ti0]()
                prev = None
                for cp in range(NCP):
                    _, kvs, vas = cur
                    nxt = make_kv_thunks(cp + 1) if cp + 1 < NCP else ([], None, None)
                    n_thunks = len(nxt[0])
                    ti = 0
                    pair_idx = 0
                    for tch in range(NTCH):
                        for pb in range(4):
                            ps = psp.tile([128, 1024], f32, tag="ps")
                            nc.tensor.matmul(
                                ps[:, 0:512],
                                kvs[0][0:64, pb * 128:(pb + 1) * 128],
                                qt2[0:64, tch * 512:(tch + 1) * 512],
                                start=True, stop=True)
                            nc.tensor.matmul(
                                ps[:, 512:1024],
                                kvs[1][64:128, pb * 128:(pb + 1) * 128],
                                qt2[64:128, tch * 512:(tch + 1) * 512],
                                start=True, stop=True)
                            u = up.tile([128, 1024], bf16, tag="u")
                            nc.scalar.activation(u[:], ps[:], Exp, scale=SCALE)
                            if prev is not None:
                                emit_pv(prev)
                            prev = (u, (vas[0], vas[1]), cp, tch, pb)
                            pair_idx += 1
                            # interleave next chunk-pair's kv work
                            target = (n_thunks * pair_idx) // 16
                            while ti < target:
                                nxt[0][ti]()
                                ti += 1
                    while ti < n_thunks:
                        nxt[0][ti]()
                        ti += 1
                    cur = nxt
                emit_pv(prev)

    nc.compile()
    return nc


def _get_nc(reps=1):
    if reps not in _CACHE:
        _CACHE[reps] = _build(reps)
    return _CACHE[reps]


def _pack_act(a, nch):
    """[L, C] row-major -> [L/512, 128, KT, 512] (chunk, partition, k, col)."""
    return np.ascontiguousarray(
        a.astype(BF16).reshape(nch, 512, KT, 128).transpose(0, 3, 2, 1))


def _pack_w(w2):
    """[C, 128] -> [128, KT, 128]."""
    return np.ascontiguousarray(
        w2.astype(BF16).reshape(KT, 128, 128).transpose(1, 0, 2))


def _prep_inputs(x, encode_out, Wq, Wk, Wv):
    x = np.asarray(x, dtype=np.float32)
    encode_out = np.asarray(encode_out, dtype=np.float32)
    Wq = np.asarray(Wq, dtype=np.float32)
    Wk = np.asarray(Wk, dtype=np.float32)
    Wv = np.asarray(Wv, dtype=np.float32)
    wqq = _pack_w(np.concatenate([Wq, Wq], axis=1))
    wkv = _pack_w(np.concatenate([Wk, Wv], axis=1))
    wvk = _pack_w(np.concatenate([Wv, Wk], axis=1))
    encTs = [_pack_act(encode_out[b], S // 512) for b in range(B)]
    in_maps = []
    for core in range(NCORE):
        b, th = divmod(core, 2)
        xTi = _pack_act(x[b, th * TSH:(th + 1) * TSH, :], NTCH)
        in_maps.append({"xT": xTi, "encT": encTs[b],
                        "Wqq": wqq, "Wkv": wkv, "Wvk": wvk})
    return in_maps


def kernel(x, encode_out, Wq, Wk, Wv):
    from concourse.bass_utils import run_bass_kernel_spmd
    nc = _get_nc(1)
    in_maps = _prep_inputs(x, encode_out, Wq, Wk, Wv)
    res = run_bass_kernel_spmd(nc, in_maps, list(range(NCORE)))
    out = np.empty((B, T, HS), dtype=np.float32)
    for core in range(NCORE):
        b, th = divmod(core, 2)
        o = np.asarray(res.results[core]["out"],
                       dtype=np.float32)       # [4, 128, 4, 64] bf16 -> f32
        out[b, th * TSH:(th + 1) * TSH] = (
            o.transpose(0, 2, 1, 3).reshape(TSH, HS))
    return out
